# revision 69
# baseline (speedup 1.0000x reference)
"""Self-contained Trainium2 Bass kernel for the "Attentive" GNN message-passing
problem:

    x: [8192, 256] f32, attn_vectors: [4, 256] f32
    e_h = l2_normalize(attn_vectors[h] * x, axis=-1)        # [H, N, D]
    out = (1/H) sum_h e_h @ e_h^T                           # [N, N]

Strategy (8 NeuronCores, SPMD, no collectives):
  - The output is SYMMETRIC: only the 136 upper-triangle 512x512 blocks of
    the 16x16 block grid are computed; the host mirrors the rest.
  - Blocks are dealt with a rotation scheme: a FIXED set S of 17 slot-pairs
    covers all 136 unordered pairs exactly once under slot -> slot+c (mod 16),
    c = core id. Every core runs the IDENTICAL program on x rolled by
    c*512 rows (host-side roll), so the program is core-agnostic.
  - Every core builds all 16 normalized/scaled panels g_p resident in SBUF:
       g[d_chunk, kc, n] = SCALE_A * attn_h[d] * x[n, d] * rnorm_h[n]
    (kc = h*2+c chunks of 128 contraction rows), then computes its 17
    blocks as plain g_i^T g_j matmuls.
  - fp8e4 (e4m3) matmuls in DoubleRow perf mode (two 128-deep k-tiles per
    instruction). g is scaled 16x up (SCALE_A=8 vs the exact 0.5) so fp8
    values sit in the normal range; the host divides the result by 256.
  - x arrives bf16 and is loaded pre-transposed through the DMA crossbar
    (no PE transposes); the shared bf16 rounding keeps rows exactly
    unit-norm after normalization.
  - g panels are built with DVE fast-path ops only (tensor_scalar at 4x,
    one wide tensor_tensor at 2x) in bf16, then converted to fp8 by a
    casting SWDGE DMA on the gpsimd queue — no engine writes fp8 directly
    (1-byte outputs force the DVE 1-elem/cycle slow path).
  - Row norms: per-panel xsq^T @ attn^2 PE matmuls; the rnorm chain runs
    one iteration behind (every op ready when issued), bounces through DRAM
    in bf16, and returns as one partition-broadcast DMA per 4-panel group.
  - The pipeline processes 4 panels per iteration (4 iterations total) to
    amortize cross-engine serialization; output blocks are staged through
    SBUF in bf16 by ACT-engine scaled copies (PSUM cannot be DMA'd) and
    shipped one DMA per block.
"""

from contextlib import ExitStack

import numpy as np

N, D, H = 8192, 256, 4
NCORES = 8
P = 128
PANEL = 512
NPANELS = N // PANEL  # 16
KCH = (H * D) // P  # 8 contraction chunks of 128
CHD = D // P  # 2 chunks per head
SUB = PANEL // P  # 4 row sub-blocks per panel
EPS = 1e-12

USE_FP8 = True
DIRECT_PSUM_DMA = False

SCALE_A = 8.0 if USE_FP8 else 0.5  # folded into a8 input
OUT_SCALE = (0.5 / SCALE_A) ** 2  # host-side (or staged-copy) factor

# Fixed slot-pair set: covers all 136 unordered panel pairs exactly once
# under (si, sj) -> (si+c, sj+c) mod 16, c = 0..7.
S_PAIRS = (
    [(0, 0)]
    + [(0, d) for d in range(1, 9)]
    + [(8, 8)]
    + [(8, 8 + d) for d in range(1, 8)]
)
S_SORTED = sorted(S_PAIRS, key=lambda s: (max(s), min(s)))
NBLK = len(S_SORTED)  # 17

_COMPILED = {}


def _build_bass():
    import concourse.bass as bass
    import concourse.tile as tile
    from concourse import bacc, mybir

    f32 = mybir.dt.float32
    bf16 = mybir.dt.bfloat16
    fp8 = mybir.dt.float8e4
    gdt = fp8 if USE_FP8 else bf16

    nc = bacc.Bacc(
        "TRN2",
        target_bir_lowering=False,
        debug=False,
        enable_asserts=False,
        num_devices=NCORES,
    )
    x_t = nc.dram_tensor("x", [N, D], bf16, kind="ExternalInput")
    # Host-precomputed functions of attn_vectors (tiny):
    #   w_sq[d, c*H+h] = attn[h, c*128+d]^2          (bf16, norm matmul rhs)
    #   a8[d, kc]      = SCALE_A*attn[h, c*128+d]    (f32, kc = h*2+c)
    ws_t = nc.dram_tensor("w_sq", [P, CHD * H], bf16, kind="ExternalInput")
    a8_t = nc.dram_tensor("a8", [P, KCH], f32, kind="ExternalInput")
    out_t = nc.dram_tensor("out", [NBLK * PANEL, PANEL], bf16, kind="ExternalOutput")

    x, out = x_t.ap(), out_t.ap()

    with tile.TileContext(nc) as tc, ExitStack() as ctx:
        consts = ctx.enter_context(tc.tile_pool(name="consts", bufs=1))
        gpool = ctx.enter_context(tc.tile_pool(name="gpool", bufs=1))
        gstage = ctx.enter_context(tc.tile_pool(name="gstage", bufs=2))
        axp = ctx.enter_context(tc.tile_pool(name="axp", bufs=1))
        xtp = ctx.enter_context(tc.tile_pool(name="xtp", bufs=4))
        sq = ctx.enter_context(tc.tile_pool(name="sq", bufs=1))
        small = ctx.enter_context(tc.tile_pool(name="small", bufs=3))
        bcp = ctx.enter_context(tc.tile_pool(name="bcp", bufs=2))
        outp = ctx.enter_context(tc.tile_pool(name="outp", bufs=2))
        dram = ctx.enter_context(tc.tile_pool(name="dram", bufs=1, space="DRAM"))
        ps_pn = ctx.enter_context(tc.tile_pool(name="ps_pn", bufs=2, space="PSUM"))
        ps_pt = ctx.enter_context(tc.tile_pool(name="ps_pt", bufs=1, space="PSUM"))
        ps_out = ctx.enter_context(tc.tile_pool(name="ps_out", bufs=5, space="PSUM"))

        from concourse.masks import make_identity

        w_sq = consts.tile([P, CHD * H], bf16)
        nc.sync.dma_start(w_sq[:], ws_t.ap()[:])
        a8 = consts.tile([P, KCH], f32)
        nc.sync.dma_start(a8[:], a8_t.ap()[:])
        identb = consts.tile([P, P], bf16)
        make_identity(nc, identb[:])
        # touch the activation table early so the lazy ~1.3us table load
        # overlaps the first panel loads instead of the first square
        warm = consts.tile([1, 2], f32)
        nc.scalar.square(warm[:, 1:2], warm[:, 0:1])

        gtiles = []  # resident per-panel g (built lazily)
        bcs = {}
        xTs = {}
        LB = 4  # panels per batched crossbar-transpose load

        def load_batch(pb):
            """Load panels 4pb..4pb+3 pre-transposed through the DMA
            crossbar (one call per c-chunk): no PE transposes, no PSUM."""
            xT4 = xtp.tile([P, CHD, LB * PANEL], bf16, tag="xT4")
            for c in range(CHD):
                nc.sync.dma_start(
                    xT4[:, c, :],
                    x[pb * LB * PANEL : (pb + 1) * LB * PANEL, c * P : (c + 1) * P],
                    transpose=True,
                )
            for k in range(LB):
                xTs[LB * pb + k] = (xT4, k * PANEL)

        def xT_slice(p, c):
            xT4, o = xTs[p]
            return xT4[:, c, o : o + PANEL]

        pns = {}
        PP = 4  # panels processed per pipeline iteration
        W = PP * PANEL

        def chainA2(m):
            """Squares + norm matmuls for panel pair (2m, 2m+1) -> pn2."""
            xT4, o = xTs[PP * m]
            xsq = sq.tile([P, CHD, W], bf16, tag="xsq")
            nc.scalar.square(xsq[:], xT4[:, :, o : o + W])
            pn = ps_pn.tile([P, PP * SUB * H], f32, tag="pn")
            for pp in range(PP):
                for i in range(SUB):
                    for c in range(CHD):
                        nc.tensor.matmul(
                            pn[:, (pp * SUB + i) * H : (pp * SUB + i + 1) * H],
                            xsq[:, c, pp * PANEL + i * P : pp * PANEL + (i + 1) * P],
                            w_sq[:, c * H : (c + 1) * H],
                            start=(c == 0),
                            stop=(c == CHD - 1),
                        )
            pns[m] = pn

        def chainB2(m):
            """rnorm chain for panel pair m (pn computed an iteration ago, so
            every op here is ready to run): clamp -> 1/x -> sqrt(bf16) ->
            transpose -> DRAM bounce -> one broadcast DMA into bcs[m]."""
            pn = pns.pop(m)
            clamped = small.tile([P, PP * SUB * H], f32, tag="clamped")
            nc.vector.tensor_scalar_max(
                clamped[:], pn[:].rearrange("q (pp i h) -> q h pp i", pp=PP, h=H), EPS
            )
            inv = small.tile([P, PP * SUB * H], f32, tag="inv")
            nc.vector.reciprocal(inv[:], clamped[:])
            rnorm = small.tile([P, PP * SUB * H], bf16, tag="rnorm")
            nc.scalar.sqrt(rnorm[:], inv[:])
            pt = ps_pt.tile([PP * SUB * H, P], bf16, tag="pt")
            nc.tensor.transpose(pt[:], rnorm[:], identb[:])
            rno = small.tile([PP * SUB * H, P], bf16, tag="rno")
            nc.vector.tensor_copy(rno[:], pt[:])
            rnd = dram.tile([PP * SUB * H, P], bf16, name=f"rnd{m}")
            nc.sync.dma_start(rnd[:], rno[:])
            # broadcast back: bc[q, h, pp, n] = rnorm_{pp,h}[n] for all q
            bc = bcp.tile([P, H, PP, PANEL], bf16, tag="bc")
            # rnd flat layout is (h, pp, i, q): the h and pp dims nest
            # contiguously so the DMA AP merges to 3 dims
            src = bass.AP(
                rnd.tensor,
                rnd.offset,
                [[0, P], [PP * PANEL, H], [PANEL, PP], [1, PANEL]],
            )
            nc.sync.dma_start(bc[:], src)
            bcs[m] = bc

        def g_build2(m):
            """axT = a8 * xT for both panels (8 double-width tensor_scalar
            ops on the DVE fast path), then per panel one wide tensor_tensor
            with the rnorm broadcast and one casting SWDGE DMA to fp8."""
            bc = bcs.pop(m)
            xT4, o = xTs[PP * m]
            axT = axp.tile([P, KCH, W], bf16, tag="axT")
            for kc in range(KCH):
                h, c = divmod(kc, CHD)
                nc.vector.tensor_scalar_mul(
                    axT[:, kc, :], xT4[:, c, o : o + W], a8[:, kc : kc + 1]
                )
            for pp in range(PP):
                p = PP * m + pp
                g = gpool.tile([P, KCH, PANEL], gdt, name=f"g{p}")
                gtiles.append(g)
                assert len(gtiles) == p + 1
                if USE_FP8:
                    gb = gstage.tile([P, KCH, PANEL], bf16, tag="gb")
                else:
                    gb = g
                in1 = bass.AP(
                    bc.tensor,
                    bc.offset + pp * PANEL,
                    [list(bc.ap[0]), [PP * PANEL, H], [0, CHD], [1, PANEL]],
                )
                nc.vector.tensor_tensor(
                    gb[:].rearrange("q (h c) n -> q h c n", h=H),
                    axT[:, :, pp * PANEL : (pp + 1) * PANEL].rearrange(
                        "q (h c) n -> q h c n", h=H
                    ),
                    in1,
                    mybir.AluOpType.mult,
                )
                if USE_FP8:
                    nc.gpsimd.dma_start(g[:], gb[:])

        def do_block(b):
            si, sj = S_SORTED[b]
            gi, gj = gtiles[si], gtiles[sj]
            ot = outp.tile([P, SUB, PANEL], bf16, tag="ot")
            for r in range(SUB):
                acc = ps_out.tile([P, PANEL], f32, tag="acc")
                if USE_FP8:
                    for kp in range(KCH // 2):
                        nc.tensor.matmul(
                            acc[:],
                            gi[:, 2 * kp : 2 * kp + 2, r * P : (r + 1) * P],
                            gj[:, 2 * kp : 2 * kp + 2, :],
                            start=(kp == 0),
                            stop=(kp == KCH // 2 - 1),
                            perf_mode=mybir.MatmulPerfMode.DoubleRow,
                        )
                else:
                    for kc in range(KCH):
                        nc.tensor.matmul(
                            acc[:],
                            gi[:, kc, r * P : (r + 1) * P],
                            gj[:, kc, :],
                            start=(kc == 0),
                            stop=(kc == KCH - 1),
                        )
                nc.scalar.mul(ot[:, r, :], acc[:], OUT_SCALE)
            dst = out[b * PANEL : (b + 1) * PANEL, :].rearrange(
                "(r q) c -> q r c", q=P
            )
            nc.sync.dma_start(dst, ot[:])

        blocks_at = {}
        for b, (si, sj) in enumerate(S_SORTED):
            blocks_at.setdefault(max(si, sj) // PP, []).append(b)

        # Software pipeline over panel PAIRS (8 iterations — the fixed
        # per-iteration serialization cost amortizes over 2 panels).
        # Iteration m emits, in order:
        #   chainB2(m+1)  — rnorm chain whose norm-matmuls ran an iteration
        #                   ago: every op is READY, never head-of-line blocks
        #   g_build2(m)   — bc(m) was issued early in iteration m-1
        #   blocks(m-1)   — g(m-1)'s casts finished mid-iteration m-1
        #   chainA2(m+2)  — squares at the ACT tail, norm-matmuls at the PE
        #                   tail (behind the ready block matmuls)
        NITER = NPANELS // PP
        NBATCH = NPANELS // LB
        nloaded = 0

        def ensure_batches(upto):
            nonlocal nloaded
            while nloaded <= min(upto, NBATCH - 1):
                load_batch(nloaded)
                nloaded += 1

        ensure_batches((PP - 1) // LB)
        chainA2(0)
        ensure_batches((2 * PP - 1) // LB + 1)
        chainA2(1)
        chainB2(0)
        for m in range(NITER):
            # keep one batch of lookahead past what chainA2(m+2) will read
            ensure_batches(((m + 3) * PP - 1) // LB + 1)
            if m + 1 < NITER:
                chainB2(m + 1)
            g_build2(m)
            for b in blocks_at.get(m, []):
                do_block(b)
            if m + 2 < NITER:
                chainA2(m + 2)

    nc.compile()
    return nc


def _get_compiled():
    if "nc" not in _COMPILED:
        _COMPILED["nc"] = _build_bass()
    return _COMPILED["nc"]


def host_side_inputs(x, attn):
    """Per-core input maps. Core c sees x rolled up by c*512 rows so the
    identical program computes a distinct set of output blocks."""
    import ml_dtypes

    w_sq = np.zeros((P, CHD * H), dtype=np.float32)
    a8 = np.zeros((P, KCH), dtype=np.float32)
    for c in range(CHD):
        w_sq[:, c * H : (c + 1) * H] = (attn[:, c * P : (c + 1) * P] ** 2).T
    for kc in range(KCH):
        h, c = divmod(kc, CHD)
        a8[:, kc] = SCALE_A * attn[h, c * P : (c + 1) * P]
    w_sq = w_sq.astype(ml_dtypes.bfloat16)
    xb = x.astype(ml_dtypes.bfloat16)
    return [
        {
            "x": np.ascontiguousarray(np.roll(xb, -c * PANEL, axis=0)),
            "w_sq": w_sq,
            "a8": a8,
        }
        for c in range(NCORES)
    ]


def assemble(results):
    """Scatter each core's 17 blocks (and their mirrors) into the full
    [N, N] output."""
    scale = OUT_SCALE if DIRECT_PSUM_DMA else 1.0
    full = np.empty((N, N), dtype=np.float32)
    for c in range(NCORES):
        o = np.asarray(results[c]["out"], dtype=np.float32)
        for b, (si, sj) in enumerate(S_SORTED):
            bi, bj = (si + c) % NPANELS, (sj + c) % NPANELS
            blk = o[b * PANEL : (b + 1) * PANEL, :]
            if scale != 1.0:
                blk = blk * scale
            if bi == bj:
                sblk = (blk + blk.T) * 0.5
                # l2-normalized rows: the diagonal is exactly 1
                np.fill_diagonal(sblk, 1.0)
                full[bi * PANEL : (bi + 1) * PANEL, bj * PANEL : (bj + 1) * PANEL] = (
                    sblk
                )
            else:
                full[bi * PANEL : (bi + 1) * PANEL, bj * PANEL : (bj + 1) * PANEL] = blk
                full[bj * PANEL : (bj + 1) * PANEL, bi * PANEL : (bi + 1) * PANEL] = (
                    blk.T
                )
    return full


def kernel(**inputs) -> np.ndarray:
    from concourse import bass_utils

    x = np.ascontiguousarray(np.asarray(inputs["x"], dtype=np.float32))
    attn = np.ascontiguousarray(np.asarray(inputs["attn_vectors"], dtype=np.float32))
    nc = _get_compiled()
    res = bass_utils.run_bass_kernel_spmd(
        nc, host_side_inputs(x, attn), core_ids=list(range(NCORES))
    )
    return assemble(res.results)


# revision 73
# speedup vs baseline: 1.0796x; 1.0796x over previous
"""Self-contained Trainium2 Bass kernel for the "Attentive" GNN message-passing
problem:

    x: [8192, 256] f32, attn_vectors: [4, 256] f32
    e_h = l2_normalize(attn_vectors[h] * x, axis=-1)        # [H, N, D]
    out = (1/H) sum_h e_h @ e_h^T                           # [N, N]

Strategy (8 NeuronCores, SPMD, no collectives):
  - The output is SYMMETRIC: only the 136 upper-triangle 512x512 blocks of
    the 16x16 block grid are computed; the host mirrors the rest.
  - Blocks are dealt with a rotation scheme: a FIXED set S of 17 slot-pairs
    covers all 136 unordered pairs exactly once under slot -> slot+c (mod 16),
    c = core id. Every core runs the IDENTICAL program on x rolled by
    c*512 rows (host-side roll), so the program is core-agnostic.
  - Every core builds all 16 normalized/scaled panels g_p resident in SBUF:
       g[d_chunk, kc, n] = SCALE_A * attn_h[d] * x[n, d] * rnorm_h[n]
    (kc = h*2+c chunks of 128 contraction rows), then computes its 17
    blocks as plain g_i^T g_j matmuls.
  - fp8e4 (e4m3) matmuls in DoubleRow perf mode (two 128-deep k-tiles per
    instruction). g is scaled 16x up (SCALE_A=8 vs the exact 0.5) so fp8
    values sit in the normal range; the host divides the result by 256.
  - x arrives bf16 and is loaded pre-transposed through the DMA crossbar
    (no PE transposes); the shared bf16 rounding keeps rows exactly
    unit-norm after normalization.
  - g panels are built with DVE fast-path ops only (tensor_scalar at 4x,
    one wide tensor_tensor at 2x) in bf16, then converted to fp8 by a
    casting SWDGE DMA on the gpsimd queue — no engine writes fp8 directly
    (1-byte outputs force the DVE 1-elem/cycle slow path).
  - Row norms: per-panel xsq^T @ attn^2 PE matmuls; the rnorm chain runs
    one iteration behind (every op ready when issued), bounces through DRAM
    in bf16, and returns as one partition-broadcast DMA per 4-panel group.
  - The pipeline processes 4 panels per iteration (4 iterations total) to
    amortize cross-engine serialization; output blocks are staged through
    SBUF in bf16 by ACT-engine scaled copies (PSUM cannot be DMA'd) and
    shipped one DMA per block.
"""

from contextlib import ExitStack

import numpy as np

N, D, H = 8192, 256, 4
NCORES = 8
P = 128
PANEL = 512
NPANELS = N // PANEL  # 16
KCH = (H * D) // P  # 8 contraction chunks of 128
CHD = D // P  # 2 chunks per head
SUB = PANEL // P  # 4 row sub-blocks per panel
EPS = 1e-12

USE_FP8 = True
DIRECT_PSUM_DMA = False

SCALE_A = 8.0 if USE_FP8 else 0.5  # folded into a8 input
OUT_SCALE = (0.5 / SCALE_A) ** 2  # host-side (or staged-copy) factor

# Fixed slot-pair set: covers all 136 unordered panel pairs exactly once
# under (si, sj) -> (si+c, sj+c) mod 16, c = 0..7.
S_PAIRS = (
    [(0, 0)]
    + [(0, d) for d in range(1, 9)]
    + [(8, 8)]
    + [(8, 8 + d) for d in range(1, 8)]
)
S_SORTED = sorted(S_PAIRS, key=lambda s: (max(s), min(s)))
NBLK = len(S_SORTED)  # 17

_COMPILED = {}


def _build_bass():
    import concourse.bass as bass
    import concourse.tile as tile
    from concourse import bacc, mybir

    f32 = mybir.dt.float32
    bf16 = mybir.dt.bfloat16
    fp8 = mybir.dt.float8e4
    gdt = fp8 if USE_FP8 else bf16

    nc = bacc.Bacc(
        "TRN2",
        target_bir_lowering=False,
        debug=False,
        enable_asserts=False,
        num_devices=NCORES,
    )
    x_t = nc.dram_tensor("x", [N, D], bf16, kind="ExternalInput")
    # Host-precomputed functions of attn_vectors (tiny):
    #   w_sq[d, c*H+h] = attn[h, c*128+d]^2          (bf16, norm matmul rhs)
    #   a8[d, kc]      = SCALE_A*attn[h, c*128+d]    (f32, kc = h*2+c)
    ws_t = nc.dram_tensor("w_sq", [P, CHD * H], bf16, kind="ExternalInput")
    a8_t = nc.dram_tensor("a8", [P, KCH], f32, kind="ExternalInput")
    out_t = nc.dram_tensor("out", [NBLK * PANEL, PANEL], bf16, kind="ExternalOutput")

    x, out = x_t.ap(), out_t.ap()

    with tile.TileContext(nc) as tc, ExitStack() as ctx:
        consts = ctx.enter_context(tc.tile_pool(name="consts", bufs=1))
        gpool = ctx.enter_context(tc.tile_pool(name="gpool", bufs=1))
        gstage = ctx.enter_context(tc.tile_pool(name="gstage", bufs=2))
        axp = ctx.enter_context(tc.tile_pool(name="axp", bufs=1))
        xtp = ctx.enter_context(tc.tile_pool(name="xtp", bufs=4))
        sq = ctx.enter_context(tc.tile_pool(name="sq", bufs=1))
        small = ctx.enter_context(tc.tile_pool(name="small", bufs=3))
        bcp = ctx.enter_context(tc.tile_pool(name="bcp", bufs=2))
        outp = ctx.enter_context(tc.tile_pool(name="outp", bufs=2))
        dram = ctx.enter_context(tc.tile_pool(name="dram", bufs=1, space="DRAM"))
        ps_pn = ctx.enter_context(tc.tile_pool(name="ps_pn", bufs=1, space="PSUM"))
        ps_pt = ctx.enter_context(tc.tile_pool(name="ps_pt", bufs=1, space="PSUM"))
        ps_out = ctx.enter_context(tc.tile_pool(name="ps_out", bufs=3, space="PSUM"))

        from concourse.masks import make_identity

        w_sq = consts.tile([P, CHD * H], bf16)
        a8 = consts.tile([P, KCH], f32)
        identb = consts.tile([P, P], bf16)
        warm = consts.tile([1, 2], f32)

        def emit_consts():
            # emitted AFTER the first crossbar loads so they don't delay the
            # critical first panel; the warm square makes the lazy ~1.3us ACT
            # table load overlap the loads instead of the first real square
            nc.sync.dma_start(w_sq[:], ws_t.ap()[:])
            nc.sync.dma_start(a8[:], a8_t.ap()[:])
            make_identity(nc, identb[:])
            nc.scalar.square(warm[:, 1:2], warm[:, 0:1])

        gtiles = []  # resident per-panel g (built lazily)
        bcs = {}
        xTs = {}
        LB = 4  # panels per batched crossbar-transpose load

        def load_batch(pb):
            """Load panels 4pb..4pb+3 pre-transposed through the DMA
            crossbar (one call per c-chunk): no PE transposes, no PSUM."""
            xT4 = xtp.tile([P, CHD, LB * PANEL], bf16, tag="xT4")
            for c in range(CHD):
                nc.sync.dma_start(
                    xT4[:, c, :],
                    x[pb * LB * PANEL : (pb + 1) * LB * PANEL, c * P : (c + 1) * P],
                    transpose=True,
                )
            for k in range(LB):
                xTs[LB * pb + k] = (xT4, k * PANEL)

        def xT_slice(p, c):
            xT4, o = xTs[p]
            return xT4[:, c, o : o + PANEL]

        pns = {}
        PP = 4  # panels processed per pipeline iteration
        W = PP * PANEL

        def chainA2(m):
            """Squares + norm matmuls for panel pair (2m, 2m+1) -> pn2."""
            xT4, o = xTs[PP * m]
            xsq = sq.tile([P, CHD, W], bf16, tag="xsq")
            nc.scalar.square(xsq[:], xT4[:, :, o : o + W])
            pn = ps_pn.tile([P, PP * SUB * H], f32, tag="pn")
            for pp in range(PP):
                for i in range(SUB):
                    for c in range(CHD):
                        nc.tensor.matmul(
                            pn[:, (pp * SUB + i) * H : (pp * SUB + i + 1) * H],
                            xsq[:, c, pp * PANEL + i * P : pp * PANEL + (i + 1) * P],
                            w_sq[:, c * H : (c + 1) * H],
                            start=(c == 0),
                            stop=(c == CHD - 1),
                        )
            pns[m] = pn

        def chainB2(m):
            """rnorm chain for panel pair m (pn computed an iteration ago, so
            every op here is ready to run): clamp -> 1/x -> sqrt(bf16) ->
            transpose -> DRAM bounce -> one broadcast DMA into bcs[m]."""
            pn = pns.pop(m)
            clamped = small.tile([P, PP * SUB * H], f32, tag="clamped")
            nc.vector.tensor_scalar_max(
                clamped[:], pn[:].rearrange("q (pp i h) -> q h pp i", pp=PP, h=H), EPS
            )
            inv = small.tile([P, PP * SUB * H], f32, tag="inv")
            nc.vector.reciprocal(inv[:], clamped[:])
            rnorm = small.tile([P, PP * SUB * H], bf16, tag="rnorm")
            nc.scalar.sqrt(rnorm[:], inv[:])
            pt = ps_pt.tile([PP * SUB * H, P], bf16, tag="pt")
            nc.tensor.transpose(pt[:], rnorm[:], identb[:])
            rno = small.tile([PP * SUB * H, P], bf16, tag="rno")
            nc.vector.tensor_copy(rno[:], pt[:])
            rnd = dram.tile([PP * SUB * H, P], bf16, name=f"rnd{m}")
            nc.sync.dma_start(rnd[:], rno[:])
            # broadcast back: bc[q, h, pp, n] = rnorm_{pp,h}[n] for all q
            bc = bcp.tile([P, H, PP, PANEL], bf16, tag="bc")
            # rnd flat layout is (h, pp, i, q): the h and pp dims nest
            # contiguously so the DMA AP merges to 3 dims
            src = bass.AP(
                rnd.tensor,
                rnd.offset,
                [[0, P], [PP * PANEL, H], [PANEL, PP], [1, PANEL]],
            )
            nc.sync.dma_start(bc[:], src)
            bcs[m] = bc

        def g_build2(m):
            """axT = a8 * xT for both panels (8 double-width tensor_scalar
            ops on the DVE fast path), then per panel one wide tensor_tensor
            with the rnorm broadcast and one casting SWDGE DMA to fp8."""
            bc = bcs.pop(m)
            xT4, o = xTs[PP * m]
            axT = axp.tile([P, KCH, W], bf16, tag="axT")
            for kc in range(KCH):
                h, c = divmod(kc, CHD)
                nc.vector.tensor_scalar_mul(
                    axT[:, kc, :], xT4[:, c, o : o + W], a8[:, kc : kc + 1]
                )
            for pp in range(PP):
                p = PP * m + pp
                g = gpool.tile([P, KCH, PANEL], gdt, name=f"g{p}")
                gtiles.append(g)
                assert len(gtiles) == p + 1
                if USE_FP8:
                    gb = gstage.tile([P, KCH, PANEL], bf16, tag="gb")
                else:
                    gb = g
                in1 = bass.AP(
                    bc.tensor,
                    bc.offset + pp * PANEL,
                    [list(bc.ap[0]), [PP * PANEL, H], [0, CHD], [1, PANEL]],
                )
                nc.vector.tensor_tensor(
                    gb[:].rearrange("q (h c) n -> q h c n", h=H),
                    axT[:, :, pp * PANEL : (pp + 1) * PANEL].rearrange(
                        "q (h c) n -> q h c n", h=H
                    ),
                    in1,
                    mybir.AluOpType.mult,
                )
                if USE_FP8:
                    nc.gpsimd.dma_start(g[:], gb[:])

        def do_block(b):
            si, sj = S_SORTED[b]
            gi, gj = gtiles[si], gtiles[sj]
            ot = outp.tile([P, SUB, PANEL], bf16, tag="ot")
            for u in range(2):
                acc = ps_out.tile([P, 2, PANEL], f32, tag="acc")
                for r2 in range(2):
                    r = 2 * u + r2
                    if USE_FP8:
                        for kp in range(KCH // 2):
                            nc.tensor.matmul(
                                acc[:, r2, :],
                                gi[:, 2 * kp : 2 * kp + 2, r * P : (r + 1) * P],
                                gj[:, 2 * kp : 2 * kp + 2, :],
                                start=(kp == 0),
                                stop=(kp == KCH // 2 - 1),
                                perf_mode=mybir.MatmulPerfMode.DoubleRow,
                            )
                    else:
                        for kc in range(KCH):
                            nc.tensor.matmul(
                                acc[:, r2, :],
                                gi[:, kc, r * P : (r + 1) * P],
                                gj[:, kc, :],
                                start=(kc == 0),
                                stop=(kc == KCH - 1),
                            )
                nc.scalar.mul(ot[:, 2 * u : 2 * u + 2, :], acc[:], OUT_SCALE)
            dst = out[b * PANEL : (b + 1) * PANEL, :].rearrange(
                "(r q) c -> q r c", q=P
            )
            nc.sync.dma_start(dst, ot[:])

        blocks_at = {}
        for b, (si, sj) in enumerate(S_SORTED):
            blocks_at.setdefault(max(si, sj) // PP, []).append(b)

        # Software pipeline over panel PAIRS (8 iterations — the fixed
        # per-iteration serialization cost amortizes over 2 panels).
        # Iteration m emits, in order:
        #   chainB2(m+1)  — rnorm chain whose norm-matmuls ran an iteration
        #                   ago: every op is READY, never head-of-line blocks
        #   g_build2(m)   — bc(m) was issued early in iteration m-1
        #   blocks(m-1)   — g(m-1)'s casts finished mid-iteration m-1
        #   chainA2(m+2)  — squares at the ACT tail, norm-matmuls at the PE
        #                   tail (behind the ready block matmuls)
        NITER = NPANELS // PP
        NBATCH = NPANELS // LB
        nloaded = 0

        def ensure_batches(upto):
            nonlocal nloaded
            while nloaded <= min(upto, NBATCH - 1):
                load_batch(nloaded)
                nloaded += 1

        ensure_batches((PP - 1) // LB)
        emit_consts()
        chainA2(0)
        ensure_batches((2 * PP - 1) // LB + 1)
        chainA2(1)
        chainB2(0)
        for m in range(NITER):
            # keep one batch of lookahead past what chainA2(m+2) will read
            ensure_batches(((m + 3) * PP - 1) // LB + 1)
            if m + 1 < NITER:
                chainB2(m + 1)
            g_build2(m)
            for b in blocks_at.get(m, []):
                do_block(b)
            if m + 2 < NITER:
                chainA2(m + 2)

    nc.compile()
    return nc


def _get_compiled():
    if "nc" not in _COMPILED:
        _COMPILED["nc"] = _build_bass()
    return _COMPILED["nc"]


def host_side_inputs(x, attn):
    """Per-core input maps. Core c sees x rolled up by c*512 rows so the
    identical program computes a distinct set of output blocks."""
    import ml_dtypes

    w_sq = np.zeros((P, CHD * H), dtype=np.float32)
    a8 = np.zeros((P, KCH), dtype=np.float32)
    for c in range(CHD):
        w_sq[:, c * H : (c + 1) * H] = (attn[:, c * P : (c + 1) * P] ** 2).T
    for kc in range(KCH):
        h, c = divmod(kc, CHD)
        a8[:, kc] = SCALE_A * attn[h, c * P : (c + 1) * P]
    w_sq = w_sq.astype(ml_dtypes.bfloat16)
    xb = x.astype(ml_dtypes.bfloat16)
    return [
        {
            "x": np.ascontiguousarray(np.roll(xb, -c * PANEL, axis=0)),
            "w_sq": w_sq,
            "a8": a8,
        }
        for c in range(NCORES)
    ]


def assemble(results):
    """Scatter each core's 17 blocks (and their mirrors) into the full
    [N, N] output."""
    scale = OUT_SCALE if DIRECT_PSUM_DMA else 1.0
    full = np.empty((N, N), dtype=np.float32)
    for c in range(NCORES):
        o = np.asarray(results[c]["out"], dtype=np.float32)
        for b, (si, sj) in enumerate(S_SORTED):
            bi, bj = (si + c) % NPANELS, (sj + c) % NPANELS
            blk = o[b * PANEL : (b + 1) * PANEL, :]
            if scale != 1.0:
                blk = blk * scale
            if bi == bj:
                sblk = (blk + blk.T) * 0.5
                # l2-normalized rows: the diagonal is exactly 1
                np.fill_diagonal(sblk, 1.0)
                full[bi * PANEL : (bi + 1) * PANEL, bj * PANEL : (bj + 1) * PANEL] = (
                    sblk
                )
            else:
                full[bi * PANEL : (bi + 1) * PANEL, bj * PANEL : (bj + 1) * PANEL] = blk
                full[bj * PANEL : (bj + 1) * PANEL, bi * PANEL : (bi + 1) * PANEL] = (
                    blk.T
                )
    return full


def kernel(**inputs) -> np.ndarray:
    from concourse import bass_utils

    x = np.ascontiguousarray(np.asarray(inputs["x"], dtype=np.float32))
    attn = np.ascontiguousarray(np.asarray(inputs["attn_vectors"], dtype=np.float32))
    nc = _get_compiled()
    res = bass_utils.run_bass_kernel_spmd(
        nc, host_side_inputs(x, attn), core_ids=list(range(NCORES))
    )
    return assemble(res.results)
